# revision 1
# baseline (speedup 1.0000x reference)
"""Trainium2 Bass kernel for nn_BaseTransformerLayer_68358699483732.

Only the aggregated (x99) branch of the reference is live: per edge type t,
    q,k,v = x99 @ Wa{q,k,v} + b       (per-head H=4, D=32)
    s     = exp(clip((k[src].q[dst]) * feat[:,0] * SCALE, -5, 5))
    wv,z  = segment_sum(v[src]*s, dst), segment_sum(s, dst)
    ah    = x99 @ Waffn + baffn + (wv/(z+1)) @ Wao + bao
    out_t = ah + LN(ah) * aln_g + aln_b
The per-vertex (x0/x1) branch is dead code and is not computed.

Sharding: nodes are partitioned contiguously across 8 cores (6250 each); edges
are routed to the core owning dst. Each core projects the full kv table
(replicated compute, bf16), its own q/ffn shard (fp32), writes bf16 gather
tables to DRAM, then gathers k|v (by src, split in two halves for int16
indices) and q (by dst) per edge with dma_gather, computes scores and
messages on DVE/ACT, and segment-sums via one-hot matmuls into PSUM.
"""

import math
from contextlib import ExitStack

import numpy as np
import ml_dtypes

import concourse.bass as bass
import concourse.bacc as bacc
import concourse.mybir as mybir
import concourse.tile as tile
from concourse.bass_utils import run_bass_kernel_spmd
from concourse.masks import make_identity

BF16 = ml_dtypes.bfloat16
F32 = np.float32

P = 128
H, D = 4, 32
OH = H * D          # 128
AIN = 256
SCALE = float(1.0 / np.sqrt(128.0))

NCORES = 8
ABLATE = set()      # timing ablations: "noblocks", "nogather", "noproj"
GROUP = 4           # chunks per batched DVE op group
SB = 2              # blocks per gather superblock


def _ceil(a, b):
    return -(-a // b)


class EdgePlan:
    """Uniform (cross-core) schedule + per-core data arrays for one edge type."""

    def __init__(self, src, dst, feat0, n_nodes, nb, half):
        self.half = half
        nblk = _ceil(nb, P)
        self.nblk = nblk
        core = dst // nb
        dstl = dst - core * nb
        blk = dstl // P
        hlf = (src >= half).astype(np.int64)

        counts = np.zeros((NCORES, nblk, 2), np.int64)
        np.add.at(counts, (core, blk, hlf), 1)
        # uniform chunk counts per (block, half): max over cores
        nch = _ceil(counts.max(axis=0), P)  # [nblk, 2] ints (numpy)
        self.nch = nch
        self.sbs = [list(range(i, min(i + SB, nblk))) for i in range(0, nblk, SB)]

        # slot layout per core: for each sb: [h=0: blocks asc][h=1: blocks asc]
        # chunk column index (in the global [P, NC] chunk-space) per (b, h)
        self.choff = np.zeros((nblk, 2), np.int64)
        # column offsets of each (sb,h) segment in chunk space
        self.sb_ch0 = []  # per sb: (ch0_h0, nch_h0, ch0_h1, nch_h1)
        c = 0
        for sbl in self.sbs:
            e = []
            for h in (0, 1):
                h0 = c
                for b in sbl:
                    self.choff[b, h] = c
                    c += int(nch[b, h])
                e += [h0, c - h0]
            self.sb_ch0.append(tuple(e))
        self.nc_total = c                      # total chunks per core
        nslot = c * P

        # slot index for every edge: appearance order is (core, sb, half, block)
        nsb = len(self.sbs)
        key = (((core * nsb + blk // SB) * 2 + hlf) * nblk) + blk
        order = np.argsort(key, kind="stable")
        ks = key[order]
        uniq, first_idx, grp_counts = np.unique(ks, return_index=True,
                                                return_counts=True)
        within = np.arange(len(order)) - np.repeat(first_idx, grp_counts)
        ub = uniq % nblk
        uh = (uniq // nblk) % 2
        base = self.choff[ub, uh] * P
        slot_sorted = np.repeat(base, grp_counts) + within
        es = order
        cores_sorted = core[es]
        # fill per-core arrays
        self.kv_idx = np.zeros((NCORES, nslot), np.int16)
        self.q_idx = np.zeros((NCORES, nslot), np.int16)
        self.dst_loc = np.full((NCORES, nslot), 255.0, F32)
        self.fval = np.zeros((NCORES, nslot), F32)
        self.kv_idx[cores_sorted, slot_sorted] = (src[es] - half * hlf[es]).astype(np.int16)
        self.q_idx[cores_sorted, slot_sorted] = dstl[es].astype(np.int16)
        self.dst_loc[cores_sorted, slot_sorted] = (dstl[es] - blk[es] * P).astype(F32)
        self.fval[cores_sorted, slot_sorted] = feat0[es] * SCALE
        self.nslot = nslot

    def tiles(self):
        """Per-core [P, NC] layouts + wrapped int16 index arrays."""
        nslot, ncc = self.nslot, self.nc_total
        dst_t = np.zeros((NCORES, P, ncc), BF16)
        f_t = np.zeros((NCORES, P, ncc), F32)
        for c in range(NCORES):
            dst_t[c] = self.dst_loc[c].reshape(ncc, P).T.astype(BF16)
            f_t[c] = self.fval[c].reshape(ncc, P).T
        # wrapped idx arrays: [128, nslot // 16]; idx j of a segment at
        # [j % 16, seg_col0 + j // 16]. Segments: kv per (sb, h); q per sb.
        # Since segments are contiguous and 128-multiple sized, a global wrap
        # of the whole slot array gives exactly the per-segment wrap layout.
        kvw = np.zeros((NCORES, P, nslot // 16), np.int16)
        qw = np.zeros((NCORES, P, nslot // 16), np.int16)
        for c in range(NCORES):
            # wrapped block [16, S], replicated 8x across the 128 partitions
            # (each GPSIMD Q7 core reads its own 16-partition stripe)
            kvw[c] = np.tile(self.kv_idx[c].reshape(nslot // 16, 16).T, (8, 1))
            qw[c] = np.tile(self.q_idx[c].reshape(nslot // 16, 16).T, (8, 1))
        return dst_t, f_t, kvw, qw


def _build_program(n_nodes, nb, plans, proj_chunk=4, repeat=1):
    """Build the SPMD bass program. plans: [EdgePlan t0, EdgePlan t1]."""
    f32 = mybir.dt.float32
    bf16 = mybir.dt.bfloat16
    i16 = mybir.dt.int16
    i32 = mybir.dt.int32

    nblk = _ceil(nb, P)
    nbp = nblk * P
    npt = _ceil(n_nodes, P)      # full-table tiles
    npp = npt * P
    half = plans[0].half

    nc = bacc.Bacc("TRN2", target_bir_lowering=False, debug=False,
                   num_devices=NCORES)

    # ---- DRAM I/O ----
    x99T_bf = nc.dram_tensor("x99T_bf", [AIN, npp], bf16, kind="ExternalInput")
    x99T_own = nc.dram_tensor("x99T_own", [AIN, nbp], f32, kind="ExternalInput")
    w_kv = nc.dram_tensor("w_kv", [AIN, 256], bf16, kind="ExternalInput")
    w_qf = nc.dram_tensor("w_qf", [AIN, 256], f32, kind="ExternalInput")
    b_kv = nc.dram_tensor("b_kv", [P, 256], f32, kind="ExternalInput")
    b_qf = nc.dram_tensor("b_qf", [P, 256], f32, kind="ExternalInput")
    wao = nc.dram_tensor("wao", [P, OH], f32, kind="ExternalInput")
    # bao | aln_g | aln_b, each row-replicated to [P, OH]
    bgb = nc.dram_tensor("bgb", [P, 3 * OH], f32, kind="ExternalInput")
    eins = []
    for t in (0, 1):
        pl = plans[t]
        eins.append({
            "kvidx": nc.dram_tensor(f"kvidx{t}", [P, pl.nslot // 16], i16, kind="ExternalInput"),
            "dstloc": nc.dram_tensor(f"dstloc{t}", [P, pl.nc_total], bf16, kind="ExternalInput"),
            "fval": nc.dram_tensor(f"fval{t}", [P, pl.nc_total], f32, kind="ExternalInput"),
        })
    outs = [nc.dram_tensor(f"out{t}", [nb, OH], f32, kind="ExternalOutput")
            for t in (0, 1)]
    kv_table = nc.dram_tensor("kv_table", [npp, 256], bf16)

    with tile.TileContext(nc) as tc, ExitStack() as ctx:
        cpool = ctx.enter_context(tc.tile_pool(name="consts", bufs=1))
        rpool = ctx.enter_context(tc.tile_pool(name="resident", bufs=1))
        gpool = ctx.enter_context(tc.tile_pool(name="gather", bufs=3))
        wpool = ctx.enter_context(tc.tile_pool(name="work", bufs=5))
        ppool = ctx.enter_context(tc.tile_pool(name="post", bufs=2))

        # ---- constants ----
        iota_i = cpool.tile([P, P], i32)
        nc.gpsimd.iota(iota_i[:], pattern=[[1, P]], base=0, channel_multiplier=0)
        iota_bf = cpool.tile([P, P], bf16)
        nc.vector.tensor_copy(iota_bf[:], iota_i[:])
        ident = cpool.tile([P, P], f32)
        make_identity(nc, ident[:])
        ident_bf = cpool.tile([P, P], bf16)
        nc.vector.tensor_copy(ident_bf[:], ident[:])
        eps_t = cpool.tile([P, 1], f32)
        nc.vector.memset(eps_t[:], 1e-5)
        wkv_sb = cpool.tile([P, AIN // P, 256], bf16)
        nc.sync.dma_start(out=wkv_sb[:], in_=w_kv[:].rearrange("(a p) o -> p a o", p=P))
        wqf_sb = cpool.tile([P, AIN // P, 256], f32)
        nc.sync.dma_start(out=wqf_sb[:], in_=w_qf[:].rearrange("(a p) o -> p a o", p=P))
        bkv_sb = cpool.tile([P, 256], f32)
        nc.sync.dma_start(out=bkv_sb[:], in_=b_kv[:])
        bqf_sb = cpool.tile([P, 256], f32)
        nc.sync.dma_start(out=bqf_sb[:], in_=b_qf[:])
        wao_sb = cpool.tile([P, OH], f32)
        nc.sync.dma_start(out=wao_sb[:], in_=wao[:])
        bgb_sb = cpool.tile([P, 3 * OH], f32)
        nc.sync.dma_start(out=bgb_sb[:], in_=bgb[:])
        bao_t = bgb_sb[:, 0:OH]
        g_t = bgb_sb[:, OH:2 * OH]
        b_t = bgb_sb[:, 2 * OH:3 * OH]
        ffn_sb = rpool.tile([P, nblk, OH], f32)
        q99_sb = rpool.tile([P, nblk, OH], bf16)

        def _emit_phases():
            # ---- phase A: projections ----
            # full kv table (bf16), proj_chunk tiles of x99T per DMA
            pa_stack = ExitStack()
            apool = pa_stack.enter_context(tc.tile_pool(name="pa_sb", bufs=3))
            apool_x = pa_stack.enter_context(tc.tile_pool(name="pa_x", bufs=3))
            aps = pa_stack.enter_context(tc.tile_pool(name="pa_ps", bufs=2, space="PSUM"))
            for it0 in range(0, npt, proj_chunk):
                ntile = min(proj_chunk, npt - it0)
                xt = apool_x.tile([P, AIN // P, proj_chunk * P], bf16, tag="xbf")
                nc.sync.dma_start(
                    out=xt[:, :, :ntile * P],
                    in_=x99T_bf[:, it0 * P:(it0 + ntile) * P]
                    .rearrange("(a p) n -> p a n", p=P))
                for it in range(it0, it0 + ntile):
                    ps = aps.tile([P, 256], f32, tag="proj")
                    for kc in range(AIN // P):
                        nc.tensor.matmul(
                            ps[:], lhsT=xt[:, kc, (it - it0) * P:(it - it0 + 1) * P],
                            rhs=wkv_sb[:, kc, :], start=(kc == 0), stop=(kc == 1))
                    kv_sb = apool.tile([P, 256], bf16, tag="kvsb")
                    nc.vector.tensor_tensor(out=kv_sb[:], in0=ps[:], in1=bkv_sb[:],
                                            op=mybir.AluOpType.add)
                    nc.scalar.dma_start(out=kv_table[it * P:(it + 1) * P, :], in_=kv_sb[:])
            # own shard q/ffn (fp32)
            for it0 in range(0, nblk, proj_chunk):
                ntile = min(proj_chunk, nblk - it0)
                xt = apool_x.tile([P, AIN // P, proj_chunk * P], f32, tag="xf32")
                nc.sync.dma_start(
                    out=xt[:, :, :ntile * P],
                    in_=x99T_own[:, it0 * P:(it0 + ntile) * P]
                    .rearrange("(a p) n -> p a n", p=P))
                for it in range(it0, it0 + ntile):
                    ps = aps.tile([P, 256], f32, tag="proj")
                    for kc in range(AIN // P):
                        nc.tensor.matmul(
                            ps[:], lhsT=xt[:, kc, (it - it0) * P:(it - it0 + 1) * P],
                            rhs=wqf_sb[:, kc, :], start=(kc == 0), stop=(kc == 1))
                    nc.vector.tensor_tensor(out=q99_sb[:, it, :], in0=ps[:, 0:OH],
                                            in1=bqf_sb[:, 0:OH], op=mybir.AluOpType.add)
                    nc.vector.tensor_tensor(out=ffn_sb[:, it, :], in0=ps[:, OH:256],
                                            in1=bqf_sb[:, OH:256], op=mybir.AluOpType.add)

            pa_stack.close()
            pb_stack = ExitStack()
            accps = pb_stack.enter_context(tc.tile_pool(name="acc_ps", bufs=2, space="PSUM"))
            bpps = pb_stack.enter_context(tc.tile_pool(name="b_ps", bufs=2, space="PSUM"))
            pps = pb_stack.enter_context(tc.tile_pool(name="post_ps", bufs=1, space="PSUM"))

            # ---- phase B: edges ----
            dst_all, f_all, ix_all = [], [], []
            for t in (0, 1):
                pl = plans[t]
                ei = eins[t]
                ncc = pl.nc_total
                dst_sb = rpool.tile([P, ncc], bf16, tag=f"dst{t}", name=f"dst{t}")
                nc.sync.dma_start(out=dst_sb[:], in_=ei["dstloc"][:])
                f_sb = rpool.tile([P, ncc], f32, tag=f"f{t}", name=f"f{t}")
                nc.sync.dma_start(out=f_sb[:], in_=ei["fval"][:])
                ixt = rpool.tile([P, pl.nslot // 16], i16, tag=f"ix{t}", name=f"ix{t}")
                nc.sync.dma_start(out=ixt[:], in_=ei["kvidx"][:])
                dst_all.append(dst_sb); f_all.append(f_sb); ix_all.append(ixt)
            for t in (0, 1):
                pl = plans[t]
                ei = eins[t]
                ncc = pl.nc_total
                dst_sb, f_sb, ixt = dst_all[t], f_all[t], ix_all[t]

                mxh = max(max(e[1], e[3]) for e in pl.sb_ch0) or 1
                for isb, sbl in enumerate(pl.sbs):
                    ch0_h0, n_h0, ch0_h1, n_h1 = pl.sb_ch0[isb]
                    nh = [n_h0, n_h1]
                    ch0 = [ch0_h0, ch0_h1]
                    kvg = [None, None]
                    for h in (0, 1):
                        if nh[h] == 0:
                            continue
                        ni = nh[h] * P
                        kvg[h] = gpool.tile([P, mxh, 256], bf16, tag=f"kvg{h}",
                                            name=f"kvg{h}")
                        src_ap = kv_table[0:half, :] if h == 0 else kv_table[half:npp, :]
                        if "nogather" in ABLATE:
                            nc.sync.dma_start(out=kvg[h][:, :1, :],
                                              in_=kv_table[0:P, :])
                            continue
                        nc.gpsimd.dma_gather(
                            out_ap=kvg[h][:, :nh[h], :], in_ap=src_ap,
                            idxs_ap=ixt[:, ch0[h] * 8:ch0[h] * 8 + ni // 16],
                            num_idxs=ni, num_idxs_reg=ni,
                            elem_size=256, single_packet=(ni <= 1024))

                    for b in sbl:
                        if "noblocks" in ABLATE:
                            continue
                        acc = accps.tile([P, 132], f32, tag="acc")
                        nchb = int(pl.nch[b, 0] + pl.nch[b, 1])
                        if nchb == 0:
                            nc.vector.memset(acc[:], 0.0)
                        done = 0
                        for h in (0, 1):
                            nbh = int(pl.nch[b, h])
                            if nbh == 0:
                                continue
                            kcol0 = int(pl.choff[b, h] - ch0[h])   # col in kvg[h]
                            scol0 = int(pl.choff[b, h])            # col in chunk space
                            for g0 in range(0, nbh, GROUP):
                                gl = min(GROUP, nbh - g0)
                                kc, sc = kcol0 + g0, scol0 + g0
                                of = wpool.tile([P, GROUP, P], bf16, tag="of")
                                ia = iota_bf[:]
                                nc.vector.tensor_tensor(
                                    out=of[:, :gl, :],
                                    in0=dst_sb[:, sc:sc + gl].to_broadcast([P, gl, P]),
                                    in1=bass.AP(ia.tensor, ia.offset,
                                                [ia.ap[0], [0, gl], ia.ap[1]]),
                                    op=mybir.AluOpType.is_equal)
                                ofme_ps = bpps.tile([P, GROUP, P], bf16, tag="ofme")
                                for cc in range(gl):
                                    nc.tensor.transpose(ofme_ps[:, cc, :], of[:, cc, :],
                                                        ident_bf[:])
                                ofme = wpool.tile([P, GROUP, P], bf16, tag="ofme_sb")
                                nc.scalar.activation(
                                    out=ofme[:, :gl, :], in_=ofme_ps[:, :gl, :],
                                    func=mybir.ActivationFunctionType.Copy)
                                qg_ps = bpps.tile([P, GROUP, OH], f32, tag="qgps")
                                for cc in range(gl):
                                    nc.tensor.matmul(
                                        qg_ps[:, cc, :], lhsT=ofme[:, cc, :],
                                        rhs=q99_sb[:, b, :], start=True, stop=True,
                                        skip_group_check=True)
                                kq = wpool.tile([P, GROUP, OH], bf16, tag="kq")
                                nc.vector.tensor_tensor(
                                    out=kq[:, :gl, :].rearrange("p c (h d) -> p c h d", h=H),
                                    in0=kvg[h][:, kc:kc + gl, 0:OH]
                                    .rearrange("p c (h d) -> p c h d", h=H),
                                    in1=qg_ps[:, :gl, :]
                                    .rearrange("p c (h d) -> p c h d", h=H),
                                    op=mybir.AluOpType.mult)
                                sraw = wpool.tile([P, GROUP, H], f32, tag="sraw")
                                nc.vector.tensor_reduce(
                                    out=sraw[:, :gl, :],
                                    in_=kq[:, :gl, :].rearrange("p c (h d) -> p c h d", h=H),
                                    axis=mybir.AxisListType.X, op=mybir.AluOpType.add)
                                nc.vector.tensor_tensor(
                                    out=sraw[:, :gl, :], in0=sraw[:, :gl, :],
                                    in1=f_sb[:, sc:sc + gl].to_broadcast([P, gl, H]),
                                    op=mybir.AluOpType.mult)
                                nc.vector.tensor_scalar(
                                    out=sraw[:, :gl, :], in0=sraw[:, :gl, :],
                                    scalar1=5.0, scalar2=-5.0,
                                    op0=mybir.AluOpType.min, op1=mybir.AluOpType.max)
                                msg = wpool.tile([P, GROUP, 132], bf16, tag="msg")
                                nc.scalar.activation(
                                    out=msg[:, :gl, OH:132], in_=sraw[:, :gl, :],
                                    func=mybir.ActivationFunctionType.Exp)
                                nc.vector.tensor_tensor(
                                    out=msg[:, :gl, 0:OH].rearrange("p c (h d) -> p c h d", h=H),
                                    in0=kvg[h][:, kc:kc + gl, OH:256]
                                    .rearrange("p c (h d) -> p c h d", h=H),
                                    in1=msg[:, :gl, OH:132].to_broadcast([P, gl, H, D]),
                                    op=mybir.AluOpType.mult)
                                for cc in range(gl):
                                    nc.tensor.matmul(
                                        acc[:], lhsT=of[:, cc, :], rhs=msg[:, cc, :],
                                        start=(done == 0), stop=(done == nchb - 1),
                                        skip_group_check=True)
                                    done += 1
                        # ---- postprocess block b ----
                        z1 = ppool.tile([P, H], f32, tag="z1")
                        nc.vector.tensor_scalar_add(z1[:], acc[:, OH:132], 1.0)
                        zr = ppool.tile([P, H], f32, tag="zr")
                        nc.vector.reciprocal(zr[:], z1[:])
                        wvz = ppool.tile([P, OH], f32, tag="wvz")
                        nc.vector.tensor_tensor(
                            out=wvz[:].rearrange("p (h d) -> p h d", h=H),
                            in0=acc[:, 0:OH].rearrange("p (h d) -> p h d", h=H),
                            in1=zr[:].to_broadcast([P, H, D]),
                            op=mybir.AluOpType.mult)
                        wvzT_ps = pps.tile([P, OH], f32, tag="postps", name="wvzT_ps")
                        nc.tensor.transpose(wvzT_ps[:], wvz[:], ident[:])
                        wvzT = ppool.tile([P, OH], f32, tag="wvzT")
                        nc.vector.tensor_copy(wvzT[:], wvzT_ps[:])
                        ao_ps = pps.tile([P, OH], f32, tag="postps", name="ao_ps")
                        nc.tensor.matmul(ao_ps[:], lhsT=wvzT[:], rhs=wao_sb[:],
                                         start=True, stop=True, skip_group_check=True)
                        ah = ppool.tile([P, OH], f32, tag="ah")
                        nc.vector.tensor_tensor(out=ah[:], in0=ao_ps[:], in1=bao_t,
                                                op=mybir.AluOpType.add)
                        nc.vector.tensor_tensor(out=ah[:], in0=ah[:], in1=ffn_sb[:, b, :],
                                                op=mybir.AluOpType.add)
                        stats = ppool.tile([P, 6], f32, tag="stats")
                        nc.vector.bn_stats(stats[:], ah[:])
                        aggr = ppool.tile([P, 2], f32, tag="aggr")
                        nc.vector.bn_aggr(aggr[:], stats[:])
                        std = ppool.tile([P, 1], f32, tag="std")
                        nc.scalar.activation(std[:], aggr[:, 1:2],
                                             mybir.ActivationFunctionType.Sqrt,
                                             bias=eps_t[:])
                        rstd = ppool.tile([P, 1], f32, tag="rstd")
                        nc.vector.reciprocal(rstd[:], std[:])
                        t1 = ppool.tile([P, OH], f32, tag="t1")
                        nc.vector.tensor_scalar(
                            out=t1[:], in0=ah[:], scalar1=aggr[:, 0:1], scalar2=rstd[:],
                            op0=mybir.AluOpType.subtract, op1=mybir.AluOpType.mult)
                        nc.vector.tensor_tensor(out=t1[:], in0=t1[:], in1=g_t,
                                                op=mybir.AluOpType.mult)
                        nc.vector.tensor_tensor(out=t1[:], in0=t1[:], in1=b_t,
                                                op=mybir.AluOpType.add)
                        ob = ppool.tile([P, OH], f32, tag="ob")
                        nc.vector.tensor_tensor(out=ob[:], in0=t1[:], in1=ah[:],
                                                op=mybir.AluOpType.add)
                        rows = min(P, nb - b * P)
                        nc.sync.dma_start(out=outs[t][b * P:b * P + rows, :],
                                          in_=ob[:rows, :])

            pb_stack.close()

        if repeat == 1:
            _emit_phases()
        else:
            with tc.For_i(0, repeat, 1):
                _emit_phases()
    nc.compile()
    return nc


def _prep_inputs(x99, weights, biases, plans, n_nodes, nb):
    """Build per-core in_maps. weights: (Waq, Wak, Wav, Wao, Waffn);
    biases: (baq, bak, bav, bao, baffn, aln_g, aln_b)."""
    Waq, Wak, Wav, Wao, Waffn = weights
    baq, bak, bav, bao, baffn, aln_g, aln_b = biases
    nblk = _ceil(nb, P)
    nbp = nblk * P
    npt = _ceil(n_nodes, P)
    npp = npt * P

    x99T = np.ascontiguousarray(x99.T)                       # [AIN, N] f32
    x99T_pad = np.zeros((AIN, npp), F32)
    x99T_pad[:, :n_nodes] = x99T
    x99T_bf = x99T_pad.astype(BF16)
    w_kv = np.concatenate([Wak, Wav], axis=1).astype(BF16)   # [AIN, 256]
    w_qf = np.concatenate([Waq, Waffn], axis=1).astype(F32)
    b_kv = np.tile(np.concatenate([bak, bav])[None, :], (P, 1)).astype(F32)
    b_qf = np.tile(np.concatenate([baq, baffn])[None, :], (P, 1)).astype(F32)
    bgb = np.tile(np.concatenate([bao, aln_g, aln_b])[None, :], (P, 1)).astype(F32)
    wao_f = Wao.astype(F32)

    per_type = []
    for pl in plans:
        per_type.append(pl.tiles())

    in_maps = []
    for c in range(NCORES):
        own = np.zeros((AIN, nbp), F32)
        own[:, :nb] = x99T[:, c * nb:(c + 1) * nb]
        m = {
            "x99T_bf": x99T_bf, "x99T_own": own,
            "w_kv": w_kv, "w_qf": w_qf, "b_kv": b_kv, "b_qf": b_qf,
            "wao": wao_f, "bgb": bgb,
        }
        for t in (0, 1):
            dst_t, f_t, kvw, qw = per_type[t]
            m[f"dstloc{t}"] = dst_t[c]
            m[f"fval{t}"] = f_t[c]
            m[f"kvidx{t}"] = kvw[c]
        in_maps.append(m)
    return in_maps


def _run(inputs, n_nodes, runner=None):
    nb = n_nodes // NCORES
    half = ((n_nodes // 2) // P) * P if n_nodes % 2 else n_nodes // 2
    plans = []
    for t in (0, 1):
        src = np.asarray(inputs[f"src{t}"])
        dst = np.asarray(inputs[f"dst{t}"])
        feat = np.asarray(inputs[f"feat{t}"])[:, 0].astype(F32)
        plans.append(EdgePlan(src, dst, feat, n_nodes, nb, half))
    nc = _build_program(n_nodes, nb, plans)
    weights = tuple(np.asarray(inputs[k], F32) for k in ("Waq", "Wak", "Wav", "Wao", "Waffn"))
    biases = tuple(np.asarray(inputs[k], F32)
                   for k in ("baq", "bak", "bav", "bao", "baffn", "aln_g", "aln_b"))
    in_maps = _prep_inputs(np.asarray(inputs["x99"], F32), weights, biases,
                           plans, n_nodes, nb)
    if runner is None:
        res = run_bass_kernel_spmd(nc, in_maps, list(range(NCORES)))
        results = res.results
    else:
        results = runner(nc, in_maps)
    out = []
    for t in (0, 1):
        full = np.concatenate([results[c][f"out{t}"] for c in range(NCORES)], axis=0)
        out.append(full.astype(F32))
    return tuple(out)


def kernel(**inputs):
    return _run(inputs, 50000)


def estimate_ns(inputs, n_nodes=50000):
    """Cost-model (TimelineSim) estimate of single-core exec time."""
    from concourse.timeline_sim import TimelineSim
    nb = n_nodes // NCORES
    half = n_nodes // 2
    plans = []
    for t in (0, 1):
        src = np.asarray(inputs[f"src{t}"])
        dst = np.asarray(inputs[f"dst{t}"])
        feat = np.asarray(inputs[f"feat{t}"])[:, 0].astype(F32)
        plans.append(EdgePlan(src, dst, feat, n_nodes, nb, half))
    nc = _build_program(n_nodes, nb, plans)
    ts = TimelineSim(nc, trace=True)
    ts.simulate()
    if ts.perfetto is not None:
        try:
            ts.perfetto.save("tlsim.pftrace")
        except Exception as e:
            print("pftrace save failed:", e)
    return float(ts.time)

